# revision 36
# baseline (speedup 1.0000x reference)
"""Causal self-attention (B=4, T=2048, C=1024, NH=16) on 8 TRN2 NeuronCores.

Sharding: core = 2*b + g  (b in 0..3 batches, g in 0..1 head-groups of 8 heads).
Each core computes the qkv projection for its 8 heads, causal flash attention,
and a partial output projection (rows g*512:(g+1)*512 of w_proj).  Host sums
the two partials per batch and adds b_proj.

v5 design (PE-roofline focused; ~240us of bf16 matmul work per core):
  layout  : host pre-rearranges every input into its SBUF tile layout
            (x^T tiled [128, tb, c, 512], weights [128, c, n]) so each DMA is
            one instruction with 4KB contiguous packets at full HBM rate.
            Weights ride the Activation HWDGE queue, x/out the SP queue.
  qTp     : [2][m][128, T] bf16; rows hp*64..+64 hold head 2m+hp's q^T, the
            other 64 rows zero.  QK uses the shared kT[m] (both heads' dims)
            as stationary; the zero q rows null the other head.
  mask    : causal mask applied PRE-exp by a tiny extra matmul per diagonal
            chunk: stationary=identity, moving=tri_neg ([128,128] const with
            -240 strictly below the diagonal) accumulated into the S psum.
            exp(-240/8) ~ 1e-13 -> harmless.  No gpsimd in the softmax chain.
  exp     : one ACT call per key-chunk PAIR ([128,2,512] across 2 PSUM banks)
            to amortize the ~352-cycle ACT startup.
  pipeline: per (qb, m) the two head chains hp0/hp1 alternate
            QK(A,p) exp(A) PV(B,p-1) QK(B,p) exp(B) PV(A,p-1), with qkv/proj
            "filler units" pumped between steps, so the PE never waits on the
            scalar engine and stays at the warm 2.4 GHz clock.
  v_pad   : [128, kc, h, 65] bf16 = [64 v-dims | ones]; PV output row 64 is
            the softmax denominator (ones-column trick).
  norm    : unnormalized y^T (bf16) scaled at the end of each (qb, m) by
            reciprocal_approx_fast + gpsimd partition broadcast.
  schedule: qkv(0); attn(0)+qkv(1); attn(1)+qkv(2); then windows 2 and 3 are
            MERGED - m-groups of qb=2 and qb=3 interleave (qb=3 only needs
            its own q projection early, which is force-flushed) so the
            scalar-engine exp load stays ~80% instead of peaking at ~98%;
            filler = qkv(3)+proj(0..2); tail = proj(3).
"""

import numpy as np

import concourse.bass as bass
import concourse.mybir as mybir
import concourse.tile as tile
from concourse import bacc
from concourse.masks import make_identity
from concourse.bass_utils import run_bass_kernel_spmd

B, T, C = 4, 2048, 1024
NH, HD = 16, 64
G = 2              # head groups (cores per batch)
HPG = NH // G      # heads per group = 8
GD = HPG * HD      # dims per group = 512
N_CORES = B * G

FP32 = mybir.dt.float32
BF16 = mybir.dt.bfloat16

NCC = C // 128      # 8 contraction chunks for the qkv projection
NMB = GD // 128     # 4 blocks of 128 qkv-dims per section (head pairs)
NTB = T // 512      # 4 T-blocks of 512
NKC = T // 128      # 16 key chunks of 128
VP = HD + 1         # v_pad columns: 64 v-dims + ones column


def build_nc():
    nc = bacc.Bacc()

    xt4 = nc.declare_dram_parameter("xt4", [128, NTB, NCC, 512], BF16, isOutput=False)
    wq = nc.declare_dram_parameter("wq", [128, NMB, NCC, 128], BF16, isOutput=False)
    wk = nc.declare_dram_parameter("wk", [128, NCC, GD], BF16, isOutput=False)
    wv = nc.declare_dram_parameter("wv", [128, NCC, GD], BF16, isOutput=False)
    bqc = nc.declare_dram_parameter("bqc", [128, NMB], FP32, isOutput=False)
    bkc = nc.declare_dram_parameter("bkc", [128, NMB], FP32, isOutput=False)
    bvb = nc.declare_dram_parameter("bvb", [128, GD], FP32, isOutput=False)
    wp = nc.declare_dram_parameter("wp", [128, NMB, C], BF16, isOutput=False)
    out = nc.declare_dram_parameter("out", [T, C], BF16, isOutput=True)

    from contextlib import ExitStack

    with tile.TileContext(nc) as tc, ExitStack() as stack:
        consts = stack.enter_context(tc.tile_pool(name="consts", bufs=1))
        persist = stack.enter_context(tc.tile_pool(name="persist", bufs=1))

        # ---- persistent activations / weights ----
        qTp = [
            [persist.tile([128, T], BF16, tag=f"qTp{hp}{m}", name=f"qTp{hp}{m}")
             for m in range(NMB)]
            for hp in range(2)
        ]
        kT_t = [persist.tile([128, T], BF16, tag=f"kT{m}", name=f"kT{m}")
                for m in range(NMB)]
        yT_t = [persist.tile([128, T], BF16, tag=f"yT{m}", name=f"yT{m}")
                for m in range(NMB)]
        v_pad = persist.tile([128, NKC, HPG, VP], BF16, tag="v_pad", name="v_pad")
        xt_t = [persist.tile([128, NCC, 512], BF16, tag=f"xt{tb}", name=f"xt{tb}")
                for tb in range(NTB)]
        xt0a = persist.tile([128, NCC // 2, 512], BF16, tag="xt0a", name="xt0a")
        xt0b = persist.tile([128, NCC // 2, 512], BF16, tag="xt0b", name="xt0b")

        def xt_sl(tb, c):
            if tb == 0:
                return xt0a[:, c] if c < NCC // 2 else xt0b[:, c - NCC // 2]
            return xt_t[tb][:, c]
        wq_t = [persist.tile([128, NCC, 128], BF16, tag=f"wqt{m}", name=f"wqt{m}")
                for m in range(NMB)]
        wk_all = persist.tile([128, NCC, GD], BF16, tag="wk_all", name="wk_all")
        wv_all = persist.tile([128, NCC, GD], BF16, tag="wv_all", name="wv_all")
        wp_all = persist.tile([128, NMB, C], BF16, tag="wp_all", name="wp_all")

        ident = consts.tile([128, 128], BF16, tag="ident")
        make_identity(nc, ident)
        # tri_neg[k, i] = -240 where i < k (strictly below diagonal), else 0
        tri_neg = consts.tile([128, 128], BF16, tag="tri_neg")
        nc.gpsimd.memset(tri_neg, -240.0)
        nc.gpsimd.affine_select(
            out=tri_neg, in_=tri_neg,
            pattern=[[-1, 128]],
            compare_op=mybir.AluOpType.is_ge,
            fill=0.0, base=-1, channel_multiplier=1,
        )

        # ---- zero-fill the dead half of each qTp; ones column of v_pad ----
        for hp in range(2):
            zbase = (1 - hp) * 64
            for m in range(NMB):
                nc.gpsimd.memset(qTp[hp][m][zbase: zbase + 64, :], 0.0)
        nc.gpsimd.memset(v_pad[:, :, :, HD: HD + 1], 1.0)

        # ---- input DMAs: weights/bias on Act queue, x on SP queue ----
        bq_col = consts.tile([128, NMB], FP32, tag="bq_col")
        bk_col = consts.tile([128, NMB], FP32, tag="bk_col")
        bv_bc = consts.tile([128, GD], FP32, tag="bv_bc")
        nc.sync.dma_start(out=xt0a, in_=xt4[:, 0, 0: NCC // 2])
        nc.sync.dma_start(out=xt0b, in_=xt4[:, 0, NCC // 2:])
        nc.sync.dma_start(out=bq_col, in_=bqc[:, :])
        nc.sync.dma_start(out=bk_col, in_=bkc[:, :])
        nc.sync.dma_start(out=bv_bc, in_=bvb[:, :])
        for tb in range(1, NTB):
            nc.sync.dma_start(out=xt_t[tb], in_=xt4[:, tb])
        for m in range(NMB):
            nc.scalar.dma_start(out=wq_t[m], in_=wq[:, m])
        nc.scalar.dma_start(out=wk_all, in_=wk[:, :, :])
        nc.scalar.dma_start(out=wv_all, in_=wv[:, :, :])
        nc.gpsimd.dma_start(out=wp_all, in_=wp[:, :, :])

        with (
            tc.tile_pool(name="pt", bufs=8) as ptpool,
            tc.tile_pool(name="dn", bufs=2) as dnpool,
            tc.tile_pool(name="rbc", bufs=2) as rbcpool,
            tc.tile_pool(name="osb", bufs=2) as osbpool,
            tc.tile_pool(name="osbh", bufs=8) as osbhpool,
            tc.tile_pool(name="sp", bufs=2, space="PSUM") as spool,
            tc.tile_pool(name="pv", bufs=2, space="PSUM") as pvpool,
            tc.tile_pool(name="fps", bufs=2, space="PSUM") as fpool,
        ):
            # ---------------- filler units (qkv projection / out proj) -----
            def q_unit(tb, m):
                def emit():
                    ps = fpool.tile([128, 512], FP32, tag="fps", name="fps")
                    for c in range(NCC):
                        nc.tensor.matmul(
                            ps, wq_t[m][:, c, :],
                            xt_sl(tb, c),
                            start=(c == 0), stop=(c == NCC - 1),
                        )
                    tcols = bass.ts(tb, 512)
                    nc.vector.tensor_scalar_add(
                        qTp[0][m][0:64, tcols], ps[0:64, :], bq_col[0:64, m: m + 1]
                    )
                    nc.vector.tensor_scalar_add(
                        qTp[1][m][64:128, tcols], ps[64:128, :],
                        bq_col[64:128, m: m + 1],
                    )
                return emit

            def k_unit(tb, m):
                def emit():
                    ps = fpool.tile([128, 512], FP32, tag="fps", name="fps")
                    for c in range(NCC):
                        nc.tensor.matmul(
                            ps, wk_all[:, c, bass.ts(m, 128)],
                            xt_sl(tb, c),
                            start=(c == 0), stop=(c == NCC - 1),
                        )
                    nc.vector.tensor_scalar_add(
                        kT_t[m][:, bass.ts(tb, 512)], ps, bk_col[:, m: m + 1]
                    )
                return emit

            def v_unit(tb, tsub):
                def emit():
                    kc = tb * 4 + tsub
                    ps = fpool.tile([128, 512], FP32, tag="fps", name="fps")
                    for c in range(NCC):
                        nc.tensor.matmul(
                            ps, xt_sl(tb, c)[:, bass.ts(tsub, 128)],
                            wv_all[:, c, :],
                            start=(c == 0), stop=(c == NCC - 1),
                        )
                    nc.vector.tensor_add(
                        v_pad[:, kc, :, 0:HD],
                        ps.rearrange("p (h d) -> p h d", h=HPG),
                        bv_bc.rearrange("p (h d) -> p h d", h=HPG),
                    )
                return emit

            proj_osb = {}

            def proj_unit(qb, tsub, nb, split_dma=False):
                def emit():
                    tb16 = qb * 4 + tsub
                    ps = fpool.tile([128, 512], FP32, tag="fps", name="fps")
                    for c in range(NMB):
                        nc.tensor.matmul(
                            ps, yT_t[c][:, bass.ts(tb16, 128)],
                            wp_all[:, c, bass.ts(nb, 512)],
                            start=(c == 0), stop=(c == NMB - 1),
                        )
                    if split_dma:
                        osb = osbhpool.tile([128, 512], BF16, tag="osbh",
                                            name="osbh")
                        nc.vector.tensor_copy(osb, ps)
                        nc.sync.dma_start(
                            out=out[bass.ts(tb16, 128), bass.ts(nb, 512)],
                            in_=osb,
                        )
                    else:
                        if nb == 0:
                            proj_osb[tb16] = osbpool.tile(
                                [128, C], BF16, tag="osb", name="osb")
                        osb = proj_osb[tb16]
                        nc.vector.tensor_copy(osb[:, bass.ts(nb, 512)], ps)
                        if nb == 1:
                            nc.sync.dma_start(
                                out=out[bass.ts(tb16, 128), :], in_=osb
                            )
                return emit

            def qkv_units(tb):
                us = []
                for m in range(NMB):
                    us.append((2.0, q_unit(tb, m)))
                for m in range(NMB):
                    us.append((2.0, k_unit(tb, m)))
                for tsub in range(4):
                    us.append((2.0, v_unit(tb, tsub)))
                return us

            def proj_units(qb, split_dma=False):
                return [(1.0, proj_unit(qb, tsub, nb, split_dma))
                        for tsub in range(4) for nb in range(2)]

            # ---------------- attention groups -----------------------------
            def chunk_geom(qb, kc):
                c_off = kc - 4 * qb
                w = 512 if c_off < 0 else 512 - 128 * c_off
                return c_off, 512 - w          # (diag offset, first live col)

            def attn_window(groups, units, flush_before=None,
                            hold_back=0):
                """groups: list of (qb, m).  units: filler closures, pumped
                evenly across halfsteps.  flush_before: {group_idx: n} force-
                flushes the first n units before that group starts.
                hold_back: leave that many trailing units unemitted and
                return them (for tail interleaving)."""
                flush_before = flush_before or {}
                keep = len(units) - hold_back
                n_hs = sum(4 * (qb + 1) for qb, _ in groups)
                total_cost = sum(c for c, _ in units[:keep])
                state = {"u": 0, "hs": 0, "cost": 0.0}

                def pump_to(k):
                    while state["u"] < min(k, keep):
                        state["cost"] += units[state["u"]][0]
                        units[state["u"]][1]()
                        state["u"] += 1

                def pump():
                    state["hs"] += 1
                    while (state["u"] < keep
                           and state["cost"] * n_hs
                           < state["hs"] * total_cost):
                        state["cost"] += units[state["u"]][0]
                        units[state["u"]][1]()
                        state["u"] += 1

                for gi, (qb, m) in enumerate(groups):
                    if gi in flush_before:
                        pump_to(flush_before[gi])
                    P = 2 * (qb + 1)
                    kcmax = 4 * (qb + 1)
                    pv = [pvpool.tile([128, 512], FP32, tag="pv", name=f"pv{_hp}")
                          for _hp in range(2)]

                    def qk_exp(hp, p):
                        S = spool.tile([128, 2, 512], FP32, tag="sp", name="S")
                        pcol0 = 512
                        masks = []
                        for j in range(2):
                            kc = 2 * p + j
                            c_off, col0 = chunk_geom(qb, kc)
                            pcol0 = min(pcol0, col0)
                            diag = c_off >= 0
                            nc.tensor.matmul(
                                S[:, j, col0:512],
                                kT_t[m][:, bass.ts(kc, 128)],
                                qTp[hp][m][:, qb * 512 + col0: qb * 512 + 512],
                                start=True, stop=not diag,
                            )
                            if diag:
                                masks.append((j, col0))
                        for j, col0 in masks:
                            nc.tensor.matmul(
                                S[:, j, col0: col0 + 128],
                                ident, tri_neg,
                                start=False, stop=True,
                            )
                        pt = ptpool.tile([128, 2, 512], BF16, tag="pt", name="pt")
                        nc.scalar.activation(
                            out=pt[:, :, pcol0:512],
                            in_=S[:, :, pcol0:512],
                            func=mybir.ActivationFunctionType.Exp,
                            scale=1.0 / float(np.sqrt(HD)),
                        )
                        return pt

                    def emit_pv(hp, p, pt):
                        for j in range(2):
                            kc = 2 * p + j
                            _, col0 = chunk_geom(qb, kc)
                            nc.tensor.matmul(
                                pv[hp][0:VP, col0:512],
                                v_pad[:, kc, 2 * m + hp, :],
                                pt[:, j, col0:512],
                                start=(kc == 0), stop=(kc == kcmax - 1),
                            )

                    def drain_norm(hp):
                        nc.vector.tensor_copy(
                            yT_t[m][bass.ts(hp, 64), bass.ts(qb, 512)],
                            pv[hp][0:HD, :],
                        )
                        den = dnpool.tile([1, 512], FP32, tag=f"den{hp}",
                                          name=f"den{hp}")
                        nc.vector.tensor_copy(den, pv[hp][HD: HD + 1, :])
                        denr = dnpool.tile([1, 512], FP32, tag=f"denr{hp}",
                                           name=f"denr{hp}")
                        nc.vector.reciprocal_approx_fast(denr, den)
                        rbc = rbcpool.tile([128, 512], FP32, tag="rbc",
                                           name="rbc")
                        nc.gpsimd.partition_broadcast(rbc, denr)
                        yt = yT_t[m][bass.ts(hp, 64), bass.ts(qb, 512)]
                        nc.vector.tensor_mul(yt, yt, rbc[bass.ts(hp, 64), :])

                    hist = [[None, None], [None, None], [None, None]]
                    for p in range(P):
                        pt_a = qk_exp(0, p)
                        if p > 2:
                            emit_pv(1, p - 3, hist[2][1])
                        pump()
                        pt_b = qk_exp(1, p)
                        if p > 2:
                            emit_pv(0, p - 3, hist[2][0])
                        pump()
                        hist = [[pt_a, pt_b]] + hist[:2]
                    for back in (2, 1):
                        if P - 1 - back >= 0:
                            emit_pv(0, P - 1 - back, hist[back][0])
                            emit_pv(1, P - 1 - back, hist[back][1])
                            pump()
                    emit_pv(0, P - 1, hist[0][0])
                    drain_norm(0)
                    emit_pv(1, P - 1, hist[0][1])
                    drain_norm(1)

                pump_to(keep)
                return units[keep:]

            # ---------------- schedule -------------------------------------
            for _, u in qkv_units(0):
                u()
            attn_window([(0, m) for m in range(NMB)], qkv_units(1))
            attn_window([(1, m) for m in range(NMB)], qkv_units(2))
            # merged windows 2+3: qb3 m-groups interleave with qb2's.
            # filler: qkv(3) first (q units, then k/v), then proj(0..2).
            w23_units = qkv_units(3) + proj_units(0) + proj_units(1) \
                + proj_units(2)
            w23_groups = [(2, 0), (2, 1), (2, 2), (3, 0),
                          (2, 3), (3, 1), (3, 2), (3, 3)]
            # qkv(3) (first 12 units) must be done before any qb3 group.
            # hold back a few filler units: they interleave with proj(3)
            # phase A below so the PE never idles (keeps HAM at full clock)
            # while the last normalize chain completes.
            leftover = attn_window(w23_groups, w23_units,
                                   flush_before={3: 12}, hold_back=14)
            # proj(3) tsub 0..2 in two phases: c=0..2 accumulates into 6
            # borrowed PSUM regions while the last normalize runs; c=3
            # finishes.  tsub3 runs as two normal units at the very end.
            s1 = spool.tile([128, 2, 512], FP32, tag="sp", name="S")
            s2 = spool.tile([128, 2, 512], FP32, tag="sp", name="S")
            p3 = [pvpool.tile([128, 512], FP32, tag="pv", name=f"pv{_hp}")
                  for _hp in range(2)]
            regions = {
                (0, 0): s1[:, 0, :], (0, 1): s1[:, 1, :],
                (1, 0): s2[:, 0, :], (1, 1): s2[:, 1, :],
                (2, 0): p3[0], (2, 1): p3[1],
            }
            lv = list(leftover)
            for tsub in range(3):
                for nb in range(2):
                    ps = regions[(tsub, nb)]
                    for c in range(NMB - 1):
                        nc.tensor.matmul(
                            ps, yT_t[c][:, bass.ts(12 + tsub, 128)],
                            wp_all[:, c, bass.ts(nb, 512)],
                            start=(c == 0), stop=False,
                        )
                    if lv:
                        lv.pop(0)[1]()
            for _, u in lv:
                u()
            for tsub in range(3):
                for nb in range(2):
                    ps = regions[(tsub, nb)]
                    nc.tensor.matmul(
                        ps, yT_t[NMB - 1][:, bass.ts(12 + tsub, 128)],
                        wp_all[:, NMB - 1, bass.ts(nb, 512)],
                        start=False, stop=True,
                    )
                    osb = osbhpool.tile([128, 512], BF16, tag="osbh",
                                        name="osbh")
                    if (tsub + nb) % 2 == 0:
                        nc.vector.tensor_copy(osb, ps)
                        nc.sync.dma_start(
                            out=out[bass.ts(12 + tsub, 128),
                                    bass.ts(nb, 512)],
                            in_=osb,
                        )
                    else:
                        nc.scalar.copy(osb, ps)
                        nc.scalar.dma_start(
                            out=out[bass.ts(12 + tsub, 128),
                                    bass.ts(nb, 512)],
                            in_=osb,
                        )
            for nb in range(2):
                ps = fpool.tile([128, 512], FP32, tag="fps", name="fps")
                for c in range(NMB):
                    nc.tensor.matmul(
                        ps, yT_t[c][:, bass.ts(15, 128)],
                        wp_all[:, c, bass.ts(nb, 512)],
                        start=(c == 0), stop=(c == NMB - 1),
                    )
                osb = osbhpool.tile([128, 512], BF16, tag="osbh",
                                    name="osbh")
                if nb == 0:
                    nc.vector.tensor_copy(osb, ps)
                    nc.sync.dma_start(
                        out=out[bass.ts(15, 128), bass.ts(nb, 512)],
                        in_=osb,
                    )
                else:
                    nc.scalar.copy(osb, ps)
                    nc.scalar.dma_start(
                        out=out[bass.ts(15, 128), bass.ts(nb, 512)],
                        in_=osb,
                    )

    nc.compile()
    return nc


_CACHE = {}


def _get_nc():
    if "nc" not in _CACHE:
        _CACHE["nc"] = build_nc()
    return _CACHE["nc"]


def _to_bf16(a):
    import ml_dtypes

    return np.ascontiguousarray(np.asarray(a, dtype=np.float32).astype(ml_dtypes.bfloat16))


def make_in_maps(x, w_qkv, b_qkv, w_proj):
    x = np.asarray(x, dtype=np.float32)
    w_qkv = np.asarray(w_qkv, dtype=np.float32)
    b_qkv = np.asarray(b_qkv, dtype=np.float32)
    w_proj = np.asarray(w_proj, dtype=np.float32)

    # x[b]^T tiled: xt4[d, tb, c, tw] = x[b][512*tb+tw, 128*c+d]
    xt4s = [
        _to_bf16(x[b].T.reshape(NCC, 128, NTB, 512).transpose(1, 2, 0, 3))
        for b in range(B)
    ]

    def w_tiles(w):  # [C, N] -> [128, C//128, N]
        n = w.shape[1]
        return _to_bf16(w.reshape(C // 128, 128, n).transpose(1, 0, 2))

    def wp_tiles(w):  # [GD, C] -> [128, GD//128, C]
        return _to_bf16(w.reshape(GD // 128, 128, C).transpose(1, 0, 2))

    in_maps = []
    for core in range(N_CORES):
        b, g = divmod(core, G)
        gs = slice(GD * g, GD * g + GD)
        bq = b_qkv[gs]
        bk = b_qkv[C + GD * g: C + GD * g + GD]
        bv = b_qkv[2 * C + GD * g: 2 * C + GD * g + GD]
        in_maps.append(
            {
                "xt4": xt4s[b],
                "wq": _to_bf16(w_qkv[:, gs].reshape(NCC, 128, NMB, 128)
                               .transpose(1, 2, 0, 3)),
                "wk": w_tiles(w_qkv[:, C + GD * g: C + GD * g + GD]),
                "wv": w_tiles(w_qkv[:, 2 * C + GD * g: 2 * C + GD * g + GD]),
                "bqc": np.ascontiguousarray(bq.reshape(NMB, 128).T),
                "bkc": np.ascontiguousarray(bk.reshape(NMB, 128).T),
                "bvb": np.ascontiguousarray(
                    np.broadcast_to(bv, (128, GD)).copy()),
                "wp": wp_tiles(w_proj[gs, :]),
            }
        )
    return in_maps


def _assemble(results, b_proj):
    y = np.empty((B, T, C), dtype=np.float32)
    for b in range(B):
        y[b] = results[G * b]["out"].astype(np.float32) \
            + results[G * b + 1]["out"].astype(np.float32)
    y += np.asarray(b_proj, dtype=np.float32)[None, None, :]
    return y


def kernel(x, w_qkv, b_qkv, w_proj, b_proj):
    nc = _get_nc()
    in_maps = make_in_maps(x, w_qkv, b_qkv, w_proj)
    res = run_bass_kernel_spmd(nc, in_maps, list(range(N_CORES)))
    return _assemble(res.results, b_proj)
